# revision 8
# baseline (speedup 1.0000x reference)
"""Sparse (relu-cosine, causal+padding-masked) attention on 8 TRN2 NeuronCores.

Contract: kernel(**inputs) takes the full unsharded inputs and returns the
full [B, S, D] output. Internally:
  - host: compact each batch's tokens to the mask-valid ones (queries and
    keys share the same validity mask, so causal structure stays exactly
    lower-triangular in compacted space and all masking disappears),
    transpose X, slice per-head-pair weights, pad to tile multiples.
  - device (SPMD, 8 cores, 2 heads per core): QKV projections, cosine
    normalization folded into V rows (1/||k||) and a per-query broadcast
    tile (1/||q||), relu(QK^T) with triangular masks only on diagonal
    tiles, context accumulation, and a partial output projection through
    this core's 128 columns of Wo.
  - host: sum the 8 partial outputs, scatter rows back to the full
    [B, S, D] layout (masked query rows are exactly zero).

Matmuls run as float32r (TF32-like, fp32 accumulate).
"""

import numpy as np

B, S, D, H = 2, 2048, 1024, 16
DH = D // H
NCORES = 8
HEADS_PER_CORE = H // NCORES  # 2
JW = HEADS_PER_CORE * DH  # 128, per-core head-dim slice width
QB = 512  # query block width (one fp32 PSUM bank)
KT = 128  # key tile (partition dim)


def _build_program(LQs, n_dblk=D // 128):
    import concourse.mybir as mybir
    import concourse.tile as tile
    from concourse import bacc
    from concourse.bass import ts

    F32 = mybir.dt.float32
    F32R = mybir.dt.float32r

    LT = sum(LQs)
    offs = [0, LQs[0]]

    nc = bacc.Bacc("TRN2", target_bir_lowering=False, debug=False,
                   num_devices=NCORES)

    XT = nc.dram_tensor("XT", [D, LT], F32R, kind="ExternalInput").ap()
    WQT = nc.dram_tensor("WQT", [D, JW], F32R, kind="ExternalInput").ap()
    WKT = nc.dram_tensor("WKT", [D, JW], F32R, kind="ExternalInput").ap()
    WVT = nc.dram_tensor("WVT", [D, JW], F32R, kind="ExternalInput").ap()
    WOT = nc.dram_tensor("WOT", [JW, D], F32R, kind="ExternalInput").ap()
    # causal diag-tile masks: CAUS[:, 384-off : 384-off+qbw], off = kt0-q0
    CAUS = nc.dram_tensor("CAUS", [128, 896], F32R, kind="ExternalInput").ap()
    # IND[j, h] = 1 if j // DH == h ; INDT is its transpose
    IND = nc.dram_tensor("IND", [JW, HEADS_PER_CORE], F32R,
                         kind="ExternalInput").ap()
    INDT = nc.dram_tensor("INDT", [HEADS_PER_CORE, JW], F32R,
                          kind="ExternalInput").ap()
    OUT = nc.dram_tensor("OUT", [LT, D], F32, kind="ExternalOutput").ap()

    EPS = 1e-12

    def col_blocks(width, bw=QB):
        blocks = []
        c = 0
        while c < width:
            w = min(bw, width - c)
            blocks.append((c, w))
            c += w
        return blocks

    with tile.TileContext(nc) as tc:
        with (
            tc.tile_pool(name="consts", bufs=1) as consts,
            tc.tile_pool(name="xt", bufs=1) as xtp,
            tc.tile_pool(name="proj", bufs=1) as projp,
            tc.tile_pool(name="work", bufs=3) as work,
            tc.tile_pool(name="att", bufs=6) as attp,
            tc.tile_pool(name="outp", bufs=3) as outp,
            tc.tile_pool(name="ps_mm", bufs=2, space="PSUM") as ps_mm,
            tc.tile_pool(name="ps_scp", bufs=4, space="PSUM") as ps_scp,
            tc.tile_pool(name="ps_ctxp", bufs=1, space="PSUM") as ps_ctxp,
        ):
            # ---- constants -------------------------------------------------
            wqt = consts.tile([128, n_dblk, JW], F32R)
            wkt = consts.tile([128, n_dblk, JW], F32R)
            wvt = consts.tile([128, n_dblk, JW], F32R)
            nc.sync.dma_start(out=wqt, in_=WQT.rearrange("(k p) j -> p k j", p=128))
            nc.sync.dma_start(out=wkt, in_=WKT.rearrange("(k p) j -> p k j", p=128))
            nc.sync.dma_start(out=wvt, in_=WVT.rearrange("(k p) j -> p k j", p=128))
            wot = consts.tile([JW, D], F32R)
            nc.sync.dma_start(out=wot, in_=WOT[:, :])
            caus = consts.tile([128, 896], F32R)
            nc.sync.dma_start(out=caus, in_=CAUS[:, :])
            ind = consts.tile([JW, HEADS_PER_CORE], F32R)
            nc.sync.dma_start(out=ind, in_=IND[:, :])
            indt = consts.tile([HEADS_PER_CORE, JW], F32R)
            nc.sync.dma_start(out=indt, in_=INDT[:, :])
            eps128 = consts.tile([128, 1], F32)
            nc.vector.memset(eps128, EPS)

            # ---- load X^T (all d-blocks resident) --------------------------
            xt = xtp.tile([128, n_dblk, LT], F32R)
            for k in range(n_dblk):
                nc.sync.dma_start(out=xt[:, k, :], in_=XT[ts(k, 128), :])

            # ---- projections: QT/KT/VT = W^T-slices applied to X^T ---------
            qt = projp.tile([JW, LT], F32R)
            kt_ = projp.tile([JW, LT], F32R)
            vt = projp.tile([JW, LT], F32)
            for (c0, w), (dst, wmat) in (
                (blk, t)
                for blk in col_blocks(LT)
                for t in ((qt, wqt), (kt_, wkt), (vt, wvt))
            ):
                ps = ps_mm.tile([JW, QB], F32, tag="mm")
                for k in range(n_dblk):
                    nc.tensor.matmul(
                        ps[:, :w], wmat[:, k, :], xt[:, k, c0:c0 + w],
                        start=(k == 0), stop=(k == n_dblk - 1),
                    )
                nc.vector.tensor_copy(dst[:, c0:c0 + w], ps[:, :w])

            # ---- squares for norms -----------------------------------------
            qsq = projp.tile([JW, LT], F32R)
            ksq = projp.tile([JW, LT], F32R)
            for c0, w in col_blocks(LT):
                nc.vector.tensor_mul(qsq[:, c0:c0 + w], qt[:, c0:c0 + w],
                                     qt[:, c0:c0 + w])
                nc.vector.tensor_mul(ksq[:, c0:c0 + w], kt_[:, c0:c0 + w],
                                     kt_[:, c0:c0 + w])

            n_ttiles = LT // 128
            # kscale[t, h] = rsqrt(sum_j ksq[j, t] over head h's j)
            ksc = projp.tile([128, n_ttiles, HEADS_PER_CORE], F32)
            for tt in range(n_ttiles):
                ps = ps_mm.tile([128, HEADS_PER_CORE], F32, tag="mm", name="ps_ksum")
                nc.tensor.matmul(ps[:, :], ksq[:, ts(tt, 128)], ind[:, :],
                                 start=True, stop=True)
                nc.scalar.activation(out=ksc[:, tt, :], in_=ps[:, :],
                                     func=mybir.ActivationFunctionType.Sqrt,
                                     bias=eps128[:, :], scale=1.0)
                nc.vector.reciprocal(out=ksc[:, tt, :], in_=ksc[:, tt, :])

            # ---- V natural (PE transpose) scaled by kscale -----------------
            ident = consts.tile([128, 128], F32)
            from concourse.masks import make_identity
            make_identity(nc, ident)
            vn = projp.tile([128, n_ttiles, JW], F32R)
            for tt in range(n_ttiles):
                ps = ps_mm.tile([128, 128], F32, tag="mm", name="ps_vtr")
                nc.tensor.transpose(ps[:, :], vt[:, ts(tt, 128)], ident)
                for h in range(HEADS_PER_CORE):
                    nc.vector.tensor_scalar_mul(
                        out=vn[:, tt, ts(h, DH)], in0=ps[:, ts(h, DH)],
                        scalar1=ksc[:, tt, h:h + 1],
                    )

            # ---- attention per batch ---------------------------------------
            ctxs = []
            for b in range(B):
                ob = offs[b]
                lq = LQs[b]
                ctx_sb = attp.tile([JW, lq], F32R, tag=f"ctx_{b}", bufs=1)
                ctxs.append(ctx_sb)
                for q0, qw in col_blocks(lq):
                    # qscale broadcast tile QSB[p, q] = rsqrt(|q|^2)[q, head(p)]
                    ps_ss = ps_mm.tile([HEADS_PER_CORE, QB], F32, tag="mm", name="ps_qsum")
                    nc.tensor.matmul(ps_ss[:, :qw], ind[:, :],
                                     qsq[:, ob + q0:ob + q0 + qw],
                                     start=True, stop=True)
                    ssq = work.tile([HEADS_PER_CORE, QB], F32R, tag="ssq")
                    nc.vector.tensor_copy(ssq[:, :qw], ps_ss[:, :qw])
                    ps_qsb = ps_mm.tile([128, QB], F32, tag="mm", name="ps_qsb")
                    nc.tensor.matmul(ps_qsb[:, :qw], indt[:, :], ssq[:, :qw],
                                     start=True, stop=True)
                    qsb = work.tile([128, QB], F32, tag="qsb")
                    nc.scalar.activation(out=qsb[:, :qw], in_=ps_qsb[:, :qw],
                                         func=mybir.ActivationFunctionType.Sqrt,
                                         bias=eps128[:, :], scale=1.0)
                    nc.vector.reciprocal(out=qsb[:, :qw], in_=qsb[:, :qw])

                    n_kt = min((q0 + qw + KT - 1) // KT, lq // KT)
                    ctx_pss = [
                        ps_ctxp.tile([DH, QB], F32, tag=f"ctx_ps{h}",
                                     name="ctx_ps")
                        for h in range(HEADS_PER_CORE)
                    ]
                    for ki in range(n_kt):
                        k0 = ki * KT
                        att_tiles = []
                        for h in range(HEADS_PER_CORE):
                            sc_ps = ps_scp.tile([128, QB], F32, tag="sc", name="sc_ps")
                            nc.tensor.matmul(
                                sc_ps[:, :qw],
                                kt_[ts(h, DH), ob + k0:ob + k0 + KT],
                                qt[ts(h, DH), ob + q0:ob + q0 + qw],
                                start=True, stop=True,
                            )
                            at = attp.tile([128, QB], F32R, tag=f"at_{h}")
                            nc.scalar.activation(
                                out=at[:, :qw], in_=sc_ps[:, :qw],
                                func=mybir.ActivationFunctionType.Relu,
                            )
                            if k0 > q0 - KT:  # diagonal tile: triangular mask
                                off = k0 - q0
                                nc.vector.tensor_mul(
                                    at[:, :qw], at[:, :qw],
                                    caus[:, 384 - off:384 - off + qw],
                                )
                            att_tiles.append(at)
                        for h in range(HEADS_PER_CORE):
                            nc.tensor.matmul(
                                ctx_pss[h][:, :qw],
                                vn[:, (ob + k0) // KT, ts(h, DH)],
                                att_tiles[h][:, :qw],
                                start=(ki == 0), stop=(ki == n_kt - 1),
                                skip_group_check=True,
                            )
                    # apply 1/|q| while copying ctx out of PSUM
                    for h in range(HEADS_PER_CORE):
                        nc.vector.tensor_mul(ctx_sb[ts(h, DH), q0:q0 + qw],
                                             ctx_pss[h][:, :qw],
                                             qsb[ts(h, DH), :qw])

            # ---- output projection (partial over this core's j-slice) ------
            for b in range(B):
                ob = offs[b]
                for tt in range(LQs[b] // 128):
                    o_sb = outp.tile([128, D], F32, tag="o_sb")
                    for ci, (c0, cw) in enumerate(col_blocks(D)):
                        ps = ps_mm.tile([128, QB], F32, tag="mm", name="ps_out")
                        nc.tensor.matmul(ps[:, :cw],
                                         ctxs[b][:, ts(tt, 128)],
                                         wot[:, c0:c0 + cw],
                                         start=True, stop=True)
                        eng = nc.vector if ci % 2 == 0 else nc.scalar
                        if eng is nc.vector:
                            nc.vector.tensor_copy(o_sb[:, c0:c0 + cw], ps[:, :cw])
                        else:
                            nc.scalar.activation(
                                out=o_sb[:, c0:c0 + cw], in_=ps[:, :cw],
                                func=mybir.ActivationFunctionType.Copy)
                    nc.sync.dma_start(out=OUT[ob + tt * 128:ob + (tt + 1) * 128, :],
                                      in_=o_sb[:, :])

    nc.compile()
    return nc


def _prepare(X, masks, Wq, Wk, Wv, Wo):
    X = np.asarray(X, dtype=np.float32)
    masks = np.asarray(masks)
    Wq = np.asarray(Wq, dtype=np.float32)
    Wk = np.asarray(Wk, dtype=np.float32)
    Wv = np.asarray(Wv, dtype=np.float32)
    Wo = np.asarray(Wo, dtype=np.float32)

    idxs = [np.where(masks[b] != 0)[0] for b in range(B)]
    LQs = [max(128, int(-(-len(ix) // 128) * 128)) for ix in idxs]
    LT = sum(LQs)
    offs = [0, LQs[0]]

    # compacted, transposed X: columns = valid tokens (zero-padded)
    XTc = np.zeros((D, LT), dtype=np.float32)
    for b in range(B):
        XTc[:, offs[b]:offs[b] + len(idxs[b])] = X[b].T[:, idxs[b]]

    caus = (np.arange(896)[None, :] - 384 >= np.arange(128)[:, None])
    caus = caus.astype(np.float32)

    nc = _build_program(LQs)

    in_maps = []
    for c in range(NCORES):
        jsl = slice(c * JW, (c + 1) * JW)
        ind = np.zeros((JW, HEADS_PER_CORE), dtype=np.float32)
        for h in range(HEADS_PER_CORE):
            ind[h * DH:(h + 1) * DH, h] = 1.0
        in_maps.append({
            "XT": XTc,
            "WQT": np.ascontiguousarray(Wq[jsl, :].T),
            "WKT": np.ascontiguousarray(Wk[jsl, :].T),
            "WVT": np.ascontiguousarray(Wv[jsl, :].T),
            "WOT": np.ascontiguousarray(Wo[:, jsl].T),
            "CAUS": caus,
            "IND": ind,
            "INDT": np.ascontiguousarray(ind.T),
        })

    return nc, in_maps, (idxs, LQs, LT, offs)


def _unshard(results, meta):
    idxs, LQs, LT, offs = meta
    partial = np.zeros((LT, D), dtype=np.float64)
    for c in range(NCORES):
        partial += results[c]["OUT"].astype(np.float64)

    out = np.zeros((B, S, D), dtype=np.float32)
    for b in range(B):
        out[b, idxs[b], :] = partial[offs[b]:offs[b] + len(idxs[b]), :].astype(
            np.float32)
    return out


def kernel(X, masks, Wq, Wk, Wv, Wo):
    from concourse.bass_utils import run_bass_kernel_spmd

    nc, in_maps, meta = _prepare(X, masks, Wq, Wk, Wv, Wo)
    res = run_bass_kernel_spmd(nc, in_maps, list(range(NCORES)))
    return _unshard(res.results, meta)


def profile_run(inputs, tmpdir=None):
    """Used by test.py: same program, run with NTFF tracing enabled."""
    from concourse.bass_utils import run_bass_kernel_spmd

    nc, in_maps, meta = _prepare(**inputs)
    res = run_bass_kernel_spmd(nc, in_maps, list(range(NCORES)), trace=True,
                               tmpdir=tmpdir)
    return res


# revision 11
# speedup vs baseline: 1.0741x; 1.0741x over previous
"""Sparse (relu-cosine, causal+padding-masked) attention on 8 TRN2 NeuronCores.

Contract: kernel(**inputs) takes the full unsharded inputs and returns the
full [B, S, D] output. Internally:
  - host: compact each batch's tokens to the mask-valid ones (queries and
    keys share the same validity mask, so causal structure stays exactly
    lower-triangular in compacted space and all masking disappears),
    transpose X, slice per-head-pair weights, pad to tile multiples.
  - device (SPMD, 8 cores, 2 heads per core): QKV projections, cosine
    normalization folded into V rows (1/||k||) and a per-query broadcast
    tile (1/||q||), relu(QK^T) with triangular masks only on diagonal
    tiles, context accumulation, and a partial output projection through
    this core's 128 columns of Wo.
  - host: sum the 8 partial outputs, scatter rows back to the full
    [B, S, D] layout (masked query rows are exactly zero).

Matmuls run as float32r (TF32-like, fp32 accumulate).
"""

import numpy as np

B, S, D, H = 2, 2048, 1024, 16
DH = D // H
NCORES = 8
HEADS_PER_CORE = H // NCORES  # 2
JW = HEADS_PER_CORE * DH  # 128, per-core head-dim slice width
QB = 512  # query block width (one fp32 PSUM bank)
KT = 128  # key tile (partition dim)


def _build_program(LQs, n_dblk=D // 128):
    import concourse.mybir as mybir
    import concourse.tile as tile
    from concourse import bacc
    from concourse.bass import ts
    from concourse.masks import make_identity

    F32 = mybir.dt.float32
    F32R = mybir.dt.float32r
    AF = mybir.ActivationFunctionType

    LT = sum(LQs)
    offs = [0, LQs[0]]
    n_ttiles = LT // 128

    nc = bacc.Bacc("TRN2", target_bir_lowering=False, debug=False,
                   num_devices=NCORES)

    XT = nc.dram_tensor("XT", [D, LT], F32R, kind="ExternalInput").ap()
    WQT = nc.dram_tensor("WQT", [D, JW], F32R, kind="ExternalInput").ap()
    WKT = nc.dram_tensor("WKT", [D, JW], F32R, kind="ExternalInput").ap()
    WVT = nc.dram_tensor("WVT", [D, JW], F32R, kind="ExternalInput").ap()
    WOT = nc.dram_tensor("WOT", [JW, D], F32R, kind="ExternalInput").ap()
    # causal diag-tile masks: CAUS[:, 384-off : 384-off+qbw], off = kt0-q0
    CAUS = nc.dram_tensor("CAUS", [128, 896], F32R, kind="ExternalInput").ap()
    # IND[j, h] = 1 if j // DH == h ; INDT is its transpose
    IND = nc.dram_tensor("IND", [JW, HEADS_PER_CORE], F32R,
                         kind="ExternalInput").ap()
    INDT = nc.dram_tensor("INDT", [HEADS_PER_CORE, JW], F32R,
                          kind="ExternalInput").ap()
    OUT = nc.dram_tensor("OUT", [LT, D], F32, kind="ExternalOutput").ap()

    EPS = 1e-12

    def col_blocks(width, bw=QB):
        blocks = []
        c = 0
        while c < width:
            w = min(bw, width - c)
            blocks.append((c, w))
            c += w
        return blocks

    with tile.TileContext(nc) as tc:
        with (
            tc.tile_pool(name="consts", bufs=1) as consts,
            tc.tile_pool(name="xt", bufs=1) as xtp,
            tc.tile_pool(name="proj", bufs=1) as projp,
            tc.tile_pool(name="work", bufs=3) as work,
            tc.tile_pool(name="att", bufs=6) as attp,
            tc.tile_pool(name="outp", bufs=3) as outp,
            tc.tile_pool(name="ps_mm", bufs=2, space="PSUM") as ps_mm,
            tc.tile_pool(name="ps_scp", bufs=4, space="PSUM") as ps_scp,
            tc.tile_pool(name="ps_ctxp", bufs=1, space="PSUM") as ps_ctxp,
        ):
            # ---- weights first (first projection matmul needs them) --------
            wqt = consts.tile([128, n_dblk, JW], F32R)
            wkt = consts.tile([128, n_dblk, JW], F32R)
            wvt = consts.tile([128, n_dblk, JW], F32R)
            nc.sync.dma_start(out=wqt, in_=WQT.rearrange("(k p) j -> p k j", p=128))
            nc.sync.dma_start(out=wkt, in_=WKT.rearrange("(k p) j -> p k j", p=128))
            nc.sync.dma_start(out=wvt, in_=WVT.rearrange("(k p) j -> p k j", p=128))

            # ---- X^T (all d-blocks resident) -------------------------------
            xt = xtp.tile([128, n_dblk, LT], F32R)
            for k in range(n_dblk):
                nc.sync.dma_start(out=xt[:, k, :], in_=XT[ts(k, 128), :])

            # ---- remaining constants ---------------------------------------
            wot = consts.tile([JW, D], F32R)
            nc.sync.dma_start(out=wot, in_=WOT[:, :])
            caus = consts.tile([128, 896], F32R)
            nc.sync.dma_start(out=caus, in_=CAUS[:, :])
            ind = consts.tile([JW, HEADS_PER_CORE], F32R)
            nc.sync.dma_start(out=ind, in_=IND[:, :])
            indt = consts.tile([HEADS_PER_CORE, JW], F32R)
            nc.sync.dma_start(out=indt, in_=INDT[:, :])
            eps128 = consts.tile([128, 1], F32)
            nc.vector.memset(eps128, EPS)
            ident = consts.tile([128, 128], F32)
            make_identity(nc, ident)

            # ---- projections: QT/KT/VT = W^T-slices applied to X^T ---------
            qt = projp.tile([JW, LT], F32R)
            kt_ = projp.tile([JW, LT], F32R)
            vt = projp.tile([JW, LT], F32)
            copy_i = 0
            for c0, w in col_blocks(LT):
                for dst, wmat in ((qt, wqt), (kt_, wkt), (vt, wvt)):
                    ps = ps_mm.tile([JW, QB], F32, tag="mm", name="ps_proj")
                    for k in range(n_dblk):
                        nc.tensor.matmul(
                            ps[:, :w], wmat[:, k, :], xt[:, k, c0:c0 + w],
                            start=(k == 0), stop=(k == n_dblk - 1),
                        )
                    if copy_i % 2 == 0:
                        nc.vector.tensor_copy(dst[:, c0:c0 + w], ps[:, :w])
                    else:
                        nc.scalar.activation(out=dst[:, c0:c0 + w],
                                             in_=ps[:, :w], func=AF.Copy)
                    copy_i += 1

            # ---- squares for norms (DVE for k, ACT for q) -------------------
            qsq = projp.tile([JW, LT], F32R)
            for c0, w in col_blocks(LT):
                nc.scalar.activation(out=qsq[:, c0:c0 + w], in_=qt[:, c0:c0 + w],
                                     func=AF.Square)

            # ---- kscale[t, h] = rsqrt(sum_j ksq[j, t] over head h) ----------
            # all t-tile sums land in one PSUM bank, then one sqrt + one recip
            NH = HEADS_PER_CORE
            ksum_ps = ps_ctxp.tile([128, n_ttiles * NH], F32, tag="ctx_ps0",
                                   name="ksum_ps")
            for tt in range(n_ttiles):
                ksq = work.tile([JW, 128], F32R, tag="ksq")
                nc.vector.tensor_mul(ksq[:, :], kt_[:, ts(tt, 128)],
                                     kt_[:, ts(tt, 128)])
                nc.tensor.matmul(ksum_ps[:, tt * NH:(tt + 1) * NH],
                                 ksq[:, :], ind[:, :],
                                 start=True, stop=True, skip_group_check=True)
            ksc = projp.tile([128, n_ttiles, NH], F32)
            nc.scalar.activation(out=ksc[:, :, :].rearrange("p a b -> p (a b)"),
                                 in_=ksum_ps[:, :], func=AF.Sqrt,
                                 bias=eps128[:, :], scale=1.0)
            nc.vector.reciprocal(out=ksc[:, :, :].rearrange("p a b -> p (a b)"),
                                 in_=ksc[:, :, :].rearrange("p a b -> p (a b)"))

            # ---- V natural (PE transpose) scaled by kscale -----------------
            vn = projp.tile([128, n_ttiles, JW], F32R)
            for tt in range(n_ttiles):
                ps = ps_mm.tile([128, 128], F32, tag="mm", name="ps_vtr")
                nc.tensor.transpose(ps[:, :], vt[:, ts(tt, 128)], ident)
                for h in range(NH):
                    nc.vector.tensor_scalar_mul(
                        out=vn[:, tt, ts(h, DH)], in0=ps[:, ts(h, DH)],
                        scalar1=ksc[:, tt, h:h + 1],
                    )

            # ---- attention + output projection, per batch / q-block --------
            for b in range(B):
                ob = offs[b]
                lq = LQs[b]
                ctx_sb = attp.tile([JW, lq], F32R, tag=f"ctx_{b}", bufs=1)
                for q0, qw in col_blocks(lq):
                    # rsqrt of per-head |q|^2 on the small [NH, qw] tile,
                    # then matmul-broadcast to QSB[p, q] = qscale[q, head(p)]
                    ps_ss = ps_mm.tile([NH, QB], F32, tag="mm", name="ps_qsum")
                    nc.tensor.matmul(ps_ss[:, :qw], ind[:, :],
                                     qsq[:, ob + q0:ob + q0 + qw],
                                     start=True, stop=True)
                    ssq = work.tile([NH, QB], F32, tag="ssq")
                    nc.scalar.activation(out=ssq[:, :qw], in_=ps_ss[:, :qw],
                                         func=AF.Sqrt, bias=eps128[:NH, :],
                                         scale=1.0)
                    ssr = work.tile([NH, QB], F32R, tag="ssr")
                    with nc.allow_low_precision(reason="f32r rounding is fine"):
                        nc.vector.reciprocal(out=ssr[:, :qw], in_=ssq[:, :qw])
                    ps_qsb = ps_mm.tile([128, QB], F32, tag="mm", name="ps_qsb")
                    nc.tensor.matmul(ps_qsb[:, :qw], indt[:, :], ssr[:, :qw],
                                     start=True, stop=True)
                    qsb = work.tile([128, QB], F32, tag="qsb")
                    nc.vector.tensor_copy(qsb[:, :qw], ps_qsb[:, :qw])

                    n_kt = min((q0 + qw + KT - 1) // KT, lq // KT)
                    ctx_pss = [
                        ps_ctxp.tile([DH, QB], F32, tag=f"ctx_ps{h}",
                                     name="ctx_ps")
                        for h in range(NH)
                    ]
                    for ki in range(n_kt):
                        k0 = ki * KT
                        att_tiles = []
                        for h in range(NH):
                            sc_ps = ps_scp.tile([128, QB], F32, tag="sc",
                                                name="sc_ps")
                            nc.tensor.matmul(
                                sc_ps[:, :qw],
                                kt_[ts(h, DH), ob + k0:ob + k0 + KT],
                                qt[ts(h, DH), ob + q0:ob + q0 + qw],
                                start=True, stop=True,
                            )
                            at = attp.tile([128, QB], F32R, tag=f"at_{h}", bufs=4)
                            if h == 0:
                                nc.scalar.activation(out=at[:, :qw],
                                                     in_=sc_ps[:, :qw],
                                                     func=AF.Relu)
                            else:
                                nc.vector.tensor_relu(at[:, :qw], sc_ps[:, :qw])
                            if k0 > q0 - KT:  # diagonal tile: triangular mask
                                off = k0 - q0
                                nc.vector.tensor_mul(
                                    at[:, :qw], at[:, :qw],
                                    caus[:, 384 - off:384 - off + qw],
                                )
                            att_tiles.append(at)
                        for h in range(NH):
                            nc.tensor.matmul(
                                ctx_pss[h][:, :qw],
                                vn[:, (ob + k0) // KT, ts(h, DH)],
                                att_tiles[h][:, :qw],
                                start=(ki == 0), stop=(ki == n_kt - 1),
                                skip_group_check=True,
                            )
                    # apply 1/|q| while copying ctx out of PSUM
                    for h in range(NH):
                        nc.vector.tensor_mul(ctx_sb[ts(h, DH), q0:q0 + qw],
                                             ctx_pss[h][:, :qw],
                                             qsb[ts(h, DH), :qw])

                    # output projection for this q-block's token tiles
                    for tt in range(q0 // 128, (q0 + qw) // 128):
                        o_sb = outp.tile([128, D], F32, tag="o_sb")
                        for ci, (c0, cw) in enumerate(col_blocks(D)):
                            ps = ps_mm.tile([128, QB], F32, tag="mm",
                                            name="ps_out")
                            nc.tensor.matmul(ps[:, :cw],
                                             ctx_sb[:, ts(tt, 128)],
                                             wot[:, c0:c0 + cw],
                                             start=True, stop=True)
                            if ci % 2 == 0:
                                nc.vector.tensor_copy(o_sb[:, c0:c0 + cw],
                                                      ps[:, :cw])
                            else:
                                nc.scalar.activation(out=o_sb[:, c0:c0 + cw],
                                                     in_=ps[:, :cw],
                                                     func=AF.Copy)
                        nc.sync.dma_start(
                            out=OUT[ob + tt * 128:ob + (tt + 1) * 128, :],
                            in_=o_sb[:, :])

    nc.compile()
    return nc


def _prepare(X, masks, Wq, Wk, Wv, Wo):
    X = np.asarray(X, dtype=np.float32)
    masks = np.asarray(masks)
    Wq = np.asarray(Wq, dtype=np.float32)
    Wk = np.asarray(Wk, dtype=np.float32)
    Wv = np.asarray(Wv, dtype=np.float32)
    Wo = np.asarray(Wo, dtype=np.float32)

    idxs = [np.where(masks[b] != 0)[0] for b in range(B)]
    LQs = [max(128, int(-(-len(ix) // 128) * 128)) for ix in idxs]
    LT = sum(LQs)
    offs = [0, LQs[0]]

    # compacted, transposed X: columns = valid tokens (zero-padded)
    XTc = np.zeros((D, LT), dtype=np.float32)
    for b in range(B):
        XTc[:, offs[b]:offs[b] + len(idxs[b])] = X[b].T[:, idxs[b]]

    caus = (np.arange(896)[None, :] - 384 >= np.arange(128)[:, None])
    caus = caus.astype(np.float32)

    nc = _build_program(LQs)

    in_maps = []
    for c in range(NCORES):
        jsl = slice(c * JW, (c + 1) * JW)
        ind = np.zeros((JW, HEADS_PER_CORE), dtype=np.float32)
        for h in range(HEADS_PER_CORE):
            ind[h * DH:(h + 1) * DH, h] = 1.0
        in_maps.append({
            "XT": XTc,
            "WQT": np.ascontiguousarray(Wq[jsl, :].T),
            "WKT": np.ascontiguousarray(Wk[jsl, :].T),
            "WVT": np.ascontiguousarray(Wv[jsl, :].T),
            "WOT": np.ascontiguousarray(Wo[:, jsl].T),
            "CAUS": caus,
            "IND": ind,
            "INDT": np.ascontiguousarray(ind.T),
        })

    return nc, in_maps, (idxs, LQs, LT, offs)


def _unshard(results, meta):
    idxs, LQs, LT, offs = meta
    partial = np.zeros((LT, D), dtype=np.float64)
    for c in range(NCORES):
        partial += results[c]["OUT"].astype(np.float64)

    out = np.zeros((B, S, D), dtype=np.float32)
    for b in range(B):
        out[b, idxs[b], :] = partial[offs[b]:offs[b] + len(idxs[b]), :].astype(
            np.float32)
    return out


def kernel(X, masks, Wq, Wk, Wv, Wo):
    from concourse.bass_utils import run_bass_kernel_spmd

    nc, in_maps, meta = _prepare(X, masks, Wq, Wk, Wv, Wo)
    res = run_bass_kernel_spmd(nc, in_maps, list(range(NCORES)))
    return _unshard(res.results, meta)


def profile_run(inputs, tmpdir=None):
    """Used by test.py: same program, run with NTFF tracing enabled."""
    from concourse.bass_utils import run_bass_kernel_spmd

    nc, in_maps, meta = _prepare(**inputs)
    res = run_bass_kernel_spmd(nc, in_maps, list(range(NCORES)), trace=True,
                               tmpdir=tmpdir)
    res.output = _unshard(res.results, meta)
    return res
